# revision 9
# baseline (speedup 1.0000x reference)
"""3-layer GCN on Trainium2, node-sharded across 8 NeuronCores.

v2 strategy (graph/data parallel per sharding hint, optimized for HBM
gather traffic — the memory-regime bottleneck):
  - Nodes sharded by contiguous range: core c owns dst rows
    [c*6272, (c+1)*6272) of a 50176-row padded node space.
  - Symmetric norm folded into the tables: the gathered table stores
    t = dinv * h, the one-hot dst-selection matrices carry exact 1.0
    entries, and dinv[dst] is applied on the aggregate.  This makes the
    tables fp8-safe (one-hot stays exact): x/t1 tables are fp8_e4m3,
    halving gather bytes vs bf16.
  - Layer 3 is transform-first: z = t2 @ W2 (64 cols) is computed
    locally, AllGathered, and layer 3 gathers 64-col fp8 z rows.
  - Self-loop messages form their own tile per window: a contiguous
    DMA from the core-local shard plus a constant identity sel —
    they never touch the random-gather path.
  - One-hot sel tiles are built once on DVE and reused by all layers.
  - AllGathers are split into 7 chunks (7 windows each) issued as the
    producing windows complete, overlapping the collective behind
    compute.  Table rows use a chunk-major layout (chunk, rank, row)
    to match the concatenation order of chunked AllGather; host
    permutes x and the gather indices accordingly.
"""

import numpy as np

N = 50000
E = 800000
F = 128
H = 128
C = 64
NCORES = 8
WIN = 128
NWIN = 392            # 392 windows of 128 dst nodes
NPAD = NWIN * WIN     # 50176
WPC = NWIN // NCORES  # 49 windows per core
NLOC = WPC * WIN      # 6272 rows per core
NCH = 1               # AllGather chunks per boundary
CHW = WPC // NCH      # 7 windows per chunk
CHROWS = CHW * WIN    # 896 rows per chunk per core

_CACHE = {}


def _perm_rows(s):
    """Global padded node id -> chunk-major table row (chunk, rank, row)."""
    c = s // NLOC
    l = s % NLOC
    k = l // CHROWS
    p = l % CHROWS
    return k * (NCORES * CHROWS) + c * CHROWS + p


def _prep_edges(edge_index):
    """Bucket the E random edges by dst window, pad to uniform tiles.

    Self-loops are NOT included here (they get a dedicated contiguous
    tile per window).  Returns per-core [128, WPC*TR] idx/dlc arrays in
    (partition=slot, col=w*TR+t) layout, plus dinv arrays.
    """
    src = np.asarray(edge_index[0], np.int64)
    dst = np.asarray(edge_index[1], np.int64)
    deg = np.zeros(NPAD, np.float32)
    deg[:N] = np.bincount(dst, minlength=N).astype(np.float32) + 1.0  # + self
    dinv = np.where(deg > 0, 1.0 / np.sqrt(np.maximum(deg, 1.0)), 0.0).astype(
        np.float32
    )

    order = np.argsort(dst, kind="stable")
    src, dst = src[order], dst[order]
    win = (dst >> 7).astype(np.int64)
    cnt = np.bincount(win, minlength=NWIN)
    TR = int(-(-cnt.max() // WIN))

    starts = np.zeros(NWIN + 1, np.int64)
    starts[1:] = np.cumsum(cnt)
    slot = np.arange(len(src), dtype=np.int64) - starts[win]

    p_src = np.zeros((NWIN, TR * WIN), np.int32)
    p_dlc = np.full((NWIN, TR * WIN), 999.0, np.float32)
    flat = win * (TR * WIN) + slot
    p_src.ravel()[flat] = _perm_rows(src).astype(np.int32)
    p_dlc.ravel()[flat] = (dst & 127).astype(np.float32)

    def to_core_layout(a):
        # [NWIN, TR*WIN] -> [c, w, t, p] -> [c, p, w, t] -> [c, 128, WPC*TR]
        a = a.reshape(NCORES, WPC, TR, WIN)
        return np.ascontiguousarray(
            a.transpose(0, 3, 1, 2).reshape(NCORES, WIN, WPC * TR)
        )

    return to_core_layout(p_src), to_core_layout(p_dlc), dinv, TR


def _build_program(TR, K=1):
    from contextlib import ExitStack

    from concourse import bacc, bass, mybir, tile

    f32 = mybir.dt.float32
    bf16 = mybir.dt.bfloat16
    fp8 = mybir.dt.float8e4
    i32 = mybir.dt.int32
    eq = mybir.AluOpType.is_equal
    mul = mybir.AluOpType.mult
    AF = mybir.ActivationFunctionType
    COLS = WPC * TR
    NT = TR + 1  # tiles per window incl. self tile

    nc = bacc.Bacc(
        "TRN2",
        target_bir_lowering=False,
        debug=False,
        enable_asserts=False,
        num_devices=NCORES,
    )

    x_t = nc.dram_tensor("x", [NPAD, F], fp8, kind="ExternalInput")
    xsw_t = nc.dram_tensor("xsw", [WIN, WPC * F], fp8, kind="ExternalInput")
    w0_t = nc.dram_tensor("w0", [F, H], bf16, kind="ExternalInput")
    w1_t = nc.dram_tensor("w1", [H, H], bf16, kind="ExternalInput")
    w2_t = nc.dram_tensor("w2", [H, C], bf16, kind="ExternalInput")
    b0_t = nc.dram_tensor("b0", [WIN, H], f32, kind="ExternalInput")
    b1c_t = nc.dram_tensor("b1c", [H, 1], f32, kind="ExternalInput")
    b2_t = nc.dram_tensor("b2", [WIN, C], f32, kind="ExternalInput")
    idx_t = nc.dram_tensor("idx", [WIN, COLS], i32, kind="ExternalInput")
    dlc_t = nc.dram_tensor("dlc", [WIN, COLS], f32, kind="ExternalInput")
    dloc_t = nc.dram_tensor("dloc", [WIN, WPC], f32, kind="ExternalInput")
    dinvB_t = nc.dram_tensor("dinvB", [WIN, NLOC], f32, kind="ExternalInput")
    out_t = nc.dram_tensor("out", [NLOC, C], f32, kind="ExternalOutput")

    with tile.TileContext(nc) as tc, ExitStack() as ctx:
        dram = ctx.enter_context(tc.tile_pool(name="dram", bufs=1, space="DRAM"))
        loc1 = dram.tile([NLOC, H], fp8, name="loc1")
        locz = dram.tile([NLOC, C], fp8, name="locz")
        # chunked AllGathers write disjoint slices from NCH collectives, which
        # the Shared-DRAM single-writer check rejects -> plain internal DRAM
        # (costs a runtime-internal staging copy, but allows overlap).
        tshared = "Shared" if NCH == 1 else "Local"
        tab1s = [
            dram.tile([NPAD, H], fp8, addr_space=tshared, name=f"tab1_{r}")
            for r in range(K)
        ]
        tabzs = [
            dram.tile([NPAD, C], fp8, addr_space=tshared, name=f"tabz_{r}")
            for r in range(K)
        ]

        const = ctx.enter_context(tc.tile_pool(name="const", bufs=1))

        def load_const(name, src_t, shape, dtype):
            s = const.tile(shape, dtype, name=name)
            nc.sync.dma_start(out=s[:], in_=src_t[:])
            return s

        w0_s = load_const("w0s", w0_t, [F, H], bf16)
        w1_s = load_const("w1s", w1_t, [H, H], bf16)
        w2_s = load_const("w2s", w2_t, [H, C], bf16)
        b0_s = load_const("b0s", b0_t, [WIN, H], f32)
        b1c_s = load_const("b1cs", b1c_t, [H, 1], f32)
        b2_s = load_const("b2s", b2_t, [WIN, C], f32)
        dloc_s = load_const("dlocs", dloc_t, [WIN, WPC], f32)
        dinvB_s = load_const("dinvBs", dinvB_t, [WIN, NLOC], f32)
        xsw_s = load_const("xsws", xsw_t, [WIN, WPC * F], fp8)

        iota_i = const.tile([WIN, WIN], i32, name="iota_i")
        nc.gpsimd.iota(iota_i[:], pattern=[[1, WIN]], base=0, channel_multiplier=0)
        iota_f = const.tile([WIN, WIN], f32, name="iota_f")
        nc.vector.tensor_copy(out=iota_f[:], in_=iota_i[:])
        iota_ci = const.tile([WIN, 1], i32, name="iota_ci")
        nc.gpsimd.iota(iota_ci[:], pattern=[[0, 1]], base=0, channel_multiplier=1)
        iota_cf = const.tile([WIN, 1], f32, name="iota_cf")
        nc.vector.tensor_copy(out=iota_cf[:], in_=iota_ci[:])
        ident = const.tile([WIN, WIN], fp8, name="ident")
        nc.vector.tensor_scalar(
            out=ident[:], in0=iota_f[:], scalar1=iota_cf[:, :1], scalar2=None, op0=eq
        )

        # one-hot sel tiles, built once, reused by every layer (and repeat)
        edge = ctx.enter_context(tc.tile_pool(name="edge", bufs=1))
        sel_all = const.tile([WIN, COLS * WIN], fp8, name="sel_all")

        t1_sb = const.tile([WIN, WPC * H], fp8, name="t1_sb")
        zres = const.tile([WIN, WPC * C], fp8, name="zres")

        gpool = ctx.enter_context(tc.tile_pool(name="gp", bufs=4))
        epil = ctx.enter_context(tc.tile_pool(name="epil", bufs=4))
        psA = ctx.enter_context(tc.tile_pool(name="psA", bufs=3, space="PSUM"))
        psB = ctx.enter_context(tc.tile_pool(name="psB", bufs=3, space="PSUM"))
        psZ = ctx.enter_context(tc.tile_pool(name="psZ", bufs=2, space="PSUM"))

        first = True
        for rep in range(K):
            tab1, tabz = tab1s[rep], tabzs[rep]
            idx_s = edge.tile([WIN, COLS], i32, tag="idx", name=f"idxs_{rep}")
            nc.sync.dma_start(out=idx_s[:], in_=idx_t[:])
            if first:
                first = False
                dlc_s = edge.tile([WIN, COLS], f32, tag="dlc", name="dlcs")
                nc.sync.dma_start(out=dlc_s[:], in_=dlc_t[:])
                for col in range(COLS):
                    nc.vector.tensor_scalar(
                        out=sel_all[:, col * WIN : (col + 1) * WIN],
                        in0=iota_f[:],
                        scalar1=dlc_s[:, col : col + 1],
                        scalar2=None,
                        op0=eq,
                    )

            # ---------- layer 1: gather fp8 x, [F, d] agg, normal epilogue ----
            for w in range(WPC):
                g = gpool.tile([WIN, TR * F], fp8, tag="g", name=f"g1_{rep}_{w}")
                nc.gpsimd.indirect_dma_start(
                    out=g[:],
                    out_offset=None,
                    in_=x_t[:],
                    in_offset=bass.IndirectOffsetOnAxis(
                        ap=idx_s[:, w * TR : (w + 1) * TR], axis=0
                    ),
                )
                agg = psA.tile([F, WIN], f32, tag="agg", name=f"agg1_{rep}_{w}")
                nc.tensor.matmul(
                    out=agg[:],
                    lhsT=xsw_s[:, w * F : (w + 1) * F],
                    rhs=ident[:],
                    start=True,
                    stop=False,
                )
                for t in range(TR):
                    col = w * TR + t
                    nc.tensor.matmul(
                        out=agg[:],
                        lhsT=g[:, t * F : (t + 1) * F],
                        rhs=sel_all[:, col * WIN : (col + 1) * WIN],
                        start=False,
                        stop=(t == TR - 1),
                    )
                aggs = epil.tile([F, WIN], bf16, tag="aggs", name=f"as1_{rep}_{w}")
                nc.vector.tensor_copy(out=aggs[:], in_=agg[:])
                h_ps = psB.tile([WIN, H], f32, tag="h", name=f"h1_{rep}_{w}")
                nc.tensor.matmul(
                    out=h_ps[:], lhsT=aggs[:], rhs=w0_s[:], start=True, stop=True
                )
                ub = epil.tile([WIN, H], f32, tag="ub", name=f"ub1_{rep}_{w}")
                nc.vector.scalar_tensor_tensor(
                    out=ub[:],
                    in0=h_ps[:],
                    scalar=dloc_s[:, w : w + 1],
                    in1=b0_s[:],
                    op0=mul,
                    op1=mybir.AluOpType.add,
                )
                sl = epil.tile([WIN, H], f32, tag="sl", name=f"sl1_{rep}_{w}")
                nc.scalar.activation(out=sl[:], in_=ub[:], func=AF.Silu)
                nc.vector.tensor_scalar(
                    out=t1_sb[:, w * H : (w + 1) * H],
                    in0=sl[:],
                    scalar1=dloc_s[:, w : w + 1],
                    scalar2=None,
                    op0=mul,
                )
                nc.sync.dma_start(
                    out=loc1[w * WIN : (w + 1) * WIN, :],
                    in_=t1_sb[:, w * H : (w + 1) * H],
                )
                if w % CHW == CHW - 1:
                    k = w // CHW
                    nc.gpsimd.collective_compute(
                        "AllGather",
                        mybir.AluOpType.bypass,
                        replica_groups=[list(range(NCORES))],
                        ins=[loc1[k * CHROWS : (k + 1) * CHROWS, :].opt()],
                        outs=[
                            tab1[
                                k * NCORES * CHROWS : (k + 1) * NCORES * CHROWS, :
                            ].opt()
                        ],
                    )

            # ---------- layer 2: gather fp8 t1, transposed epilogue, z ----
            for w in range(WPC):
                g = gpool.tile([WIN, TR * F], fp8, tag="g", name=f"g2_{rep}_{w}")
                nc.gpsimd.indirect_dma_start(
                    out=g[:],
                    out_offset=None,
                    in_=tab1[:],
                    in_offset=bass.IndirectOffsetOnAxis(
                        ap=idx_s[:, w * TR : (w + 1) * TR], axis=0
                    ),
                )
                agg = psA.tile([F, WIN], f32, tag="agg", name=f"agg2_{rep}_{w}")
                nc.tensor.matmul(
                    out=agg[:],
                    lhsT=t1_sb[:, w * H : (w + 1) * H],
                    rhs=ident[:],
                    start=True,
                    stop=False,
                )
                for t in range(TR):
                    col = w * TR + t
                    nc.tensor.matmul(
                        out=agg[:],
                        lhsT=g[:, t * F : (t + 1) * F],
                        rhs=sel_all[:, col * WIN : (col + 1) * WIN],
                        start=False,
                        stop=(t == TR - 1),
                    )
                aggs = epil.tile([F, WIN], bf16, tag="aggs", name=f"as2_{rep}_{w}")
                nc.vector.tensor_copy(out=aggs[:], in_=agg[:])
                # transposed dense: hT[fo, d] = sum_f W1[f, fo] * aggs[f, d]
                hT_ps = psB.tile([H, WIN], f32, tag="h", name=f"h2_{rep}_{w}")
                nc.tensor.matmul(
                    out=hT_ps[:], lhsT=w1_s[:], rhs=aggs[:], start=True, stop=True
                )
                ua = epil.tile([H, WIN], f32, tag="ua", name=f"ua2_{rep}_{w}")
                nc.vector.tensor_mul(
                    out=ua[:],
                    in0=hT_ps[:],
                    in1=dinvB_s[:, w * WIN : (w + 1) * WIN],
                )
                sl = epil.tile([H, WIN], f32, tag="sl", name=f"sl2_{rep}_{w}")
                nc.scalar.activation(
                    out=sl[:], in_=ua[:], func=AF.Silu, bias=b1c_s[:, :1], scale=1.0
                )
                t2T = epil.tile([H, WIN], bf16, tag="t2", name=f"t2_{rep}_{w}")
                nc.vector.tensor_mul(
                    out=t2T[:],
                    in0=sl[:],
                    in1=dinvB_s[:, w * WIN : (w + 1) * WIN],
                )
                z_ps = psZ.tile([WIN, C], f32, tag="z", name=f"z_{rep}_{w}")
                nc.tensor.matmul(
                    out=z_ps[:], lhsT=t2T[:], rhs=w2_s[:], start=True, stop=True
                )
                nc.scalar.activation(
                    out=zres[:, w * C : (w + 1) * C], in_=z_ps[:], func=AF.Copy
                )
                nc.sync.dma_start(
                    out=locz[w * WIN : (w + 1) * WIN, :],
                    in_=zres[:, w * C : (w + 1) * C],
                )
                if w % CHW == CHW - 1:
                    k = w // CHW
                    nc.gpsimd.collective_compute(
                        "AllGather",
                        mybir.AluOpType.bypass,
                        replica_groups=[list(range(NCORES))],
                        ins=[locz[k * CHROWS : (k + 1) * CHROWS, :].opt()],
                        outs=[
                            tabz[
                                k * NCORES * CHROWS : (k + 1) * NCORES * CHROWS, :
                            ].opt()
                        ],
                    )

            # ---------- layer 3: gather fp8 z (64 cols), [d, c] agg ----
            for w in range(WPC):
                g = gpool.tile([WIN, TR * C], fp8, tag="g3", name=f"g3_{rep}_{w}")
                nc.gpsimd.indirect_dma_start(
                    out=g[:],
                    out_offset=None,
                    in_=tabz[:],
                    in_offset=bass.IndirectOffsetOnAxis(
                        ap=idx_s[:, w * TR : (w + 1) * TR], axis=0
                    ),
                )
                agg = psZ.tile([WIN, C], f32, tag="z", name=f"agg3_{rep}_{w}")
                nc.tensor.matmul(
                    out=agg[:],
                    lhsT=ident[:],
                    rhs=zres[:, w * C : (w + 1) * C],
                    start=True,
                    stop=False,
                )
                for t in range(TR):
                    col = w * TR + t
                    nc.tensor.matmul(
                        out=agg[:],
                        lhsT=sel_all[:, col * WIN : (col + 1) * WIN],
                        rhs=g[:, t * C : (t + 1) * C],
                        start=False,
                        stop=(t == TR - 1),
                    )
                ls = epil.tile([WIN, C], f32, tag="ls", name=f"ls_{rep}_{w}")
                nc.vector.scalar_tensor_tensor(
                    out=ls[:],
                    in0=agg[:],
                    scalar=dloc_s[:, w : w + 1],
                    in1=b2_s[:],
                    op0=mul,
                    op1=mybir.AluOpType.add,
                )
                mx = epil.tile([WIN, 1], f32, tag="mx", name=f"mx_{rep}_{w}")
                nc.vector.tensor_reduce(
                    out=mx[:],
                    in_=ls[:],
                    axis=mybir.AxisListType.X,
                    op=mybir.AluOpType.max,
                )
                nmx = epil.tile([WIN, 1], f32, tag="nmx", name=f"nmx_{rep}_{w}")
                nc.vector.tensor_scalar_mul(nmx[:], mx[:], -1.0)
                ex = epil.tile([WIN, C], f32, tag="ex", name=f"ex_{rep}_{w}")
                sm = epil.tile([WIN, 1], f32, tag="sm", name=f"sm_{rep}_{w}")
                nc.scalar.activation(
                    out=ex[:],
                    in_=ls[:],
                    func=AF.Exp,
                    bias=nmx[:, :1],
                    scale=1.0,
                    accum_out=sm[:],
                )
                lg = epil.tile([WIN, 1], f32, tag="lg", name=f"lg_{rep}_{w}")
                nc.scalar.activation(out=lg[:], in_=sm[:], func=AF.Ln)
                lse = epil.tile([WIN, 1], f32, tag="lse", name=f"lse_{rep}_{w}")
                nc.vector.tensor_add(out=lse[:], in0=lg[:], in1=mx[:])
                o = epil.tile([WIN, C], f32, tag="o", name=f"o_{rep}_{w}")
                nc.vector.tensor_scalar_sub(o[:], ls[:], lse[:, :1])
                nc.sync.dma_start(out=out_t[w * WIN : (w + 1) * WIN, :], in_=o[:])

    nc.compile()
    return nc


def _get_program(TR, K=1):
    key = (TR, K, NCH)
    if key not in _CACHE:
        _CACHE[key] = _build_program(TR, K)
    return _CACHE[key]


def _prep_inputs(x, edge_index, W0, b0, W1, b1, W2, b2):
    import ml_dtypes

    fp8 = ml_dtypes.float8_e4m3
    x = np.asarray(x, np.float32)
    idx, dlc, dinv, TR = _prep_edges(np.asarray(edge_index))

    x_pad = np.zeros((NPAD, F), np.float32)
    x_pad[:N] = x
    t0 = dinv[:, None] * x_pad
    t0_perm = np.empty((NPAD, F), fp8)
    t0_perm[_perm_rows(np.arange(NPAD))] = t0.astype(fp8)
    # pre-swizzled self table: xsw[c, p, w*F:(w+1)*F] = t0[c, w*128+p, :]
    t0_loc = t0.astype(fp8).reshape(NCORES, WPC, WIN, F)
    xsw = np.ascontiguousarray(t0_loc.transpose(0, 2, 1, 3).reshape(NCORES, WIN, WPC * F))

    dinv_loc = dinv.reshape(NCORES, WPC, WIN)  # [c, w, d]
    dloc = np.ascontiguousarray(dinv_loc.transpose(0, 2, 1))  # [c, d, w]
    dinvB = np.ascontiguousarray(
        np.broadcast_to(
            dinv.reshape(NCORES, 1, NLOC), (NCORES, WIN, NLOC)
        ).astype(np.float32)
    )

    common = {
        "x": t0_perm,
        "w0": np.asarray(W0, np.float32).astype(ml_dtypes.bfloat16),
        "w1": np.asarray(W1, np.float32).astype(ml_dtypes.bfloat16),
        "w2": np.asarray(W2, np.float32).astype(ml_dtypes.bfloat16),
        "b0": np.broadcast_to(np.asarray(b0, np.float32), (WIN, H)).copy(),
        "b1c": np.asarray(b1, np.float32).reshape(H, 1).copy(),
        "b2": np.broadcast_to(np.asarray(b2, np.float32), (WIN, C)).copy(),
    }
    in_maps = [
        dict(
            common,
            xsw=xsw[c],
            idx=idx[c],
            dlc=dlc[c],
            dloc=dloc[c],
            dinvB=dinvB[c],
        )
        for c in range(NCORES)
    ]
    return in_maps, TR


def kernel(x, edge_index, W0, b0, W1, b1, W2, b2, **_):
    from concourse.bass_utils import run_bass_kernel_spmd

    in_maps, TR = _prep_inputs(x, edge_index, W0, b0, W1, b1, W2, b2)
    nc = _get_program(TR)
    res = run_bass_kernel_spmd(nc, in_maps, list(range(NCORES)))
    out = np.concatenate(
        [np.asarray(res.results[c]["out"]) for c in range(NCORES)], axis=0
    )
    return out[:N]


# revision 10
# speedup vs baseline: 2.4899x; 2.4899x over previous
"""3-layer GCN on Trainium2, node-sharded across 8 NeuronCores.

v2 strategy (graph/data parallel per sharding hint, optimized for HBM
gather traffic — the memory-regime bottleneck):
  - Nodes sharded by contiguous range: core c owns dst rows
    [c*6272, (c+1)*6272) of a 50176-row padded node space.
  - Symmetric norm folded into the tables: the gathered table stores
    t = dinv * h, the one-hot dst-selection matrices carry exact 1.0
    entries, and dinv[dst] is applied on the aggregate.  This makes the
    tables fp8-safe (one-hot stays exact): x/t1 tables are fp8_e4m3,
    halving gather bytes vs bf16.
  - Layer 3 is transform-first: z = t2 @ W2 (64 cols) is computed
    locally, AllGathered, and layer 3 gathers 64-col fp8 z rows.
  - Self-loop messages form their own tile per window: a contiguous
    DMA from the core-local shard plus a constant identity sel —
    they never touch the random-gather path.
  - One-hot sel tiles are built once on DVE and reused by all layers.
  - AllGathers are split into 7 chunks (7 windows each) issued as the
    producing windows complete, overlapping the collective behind
    compute.  Table rows use a chunk-major layout (chunk, rank, row)
    to match the concatenation order of chunked AllGather; host
    permutes x and the gather indices accordingly.
"""

import numpy as np

N = 50000
E = 800000
F = 128
H = 128
C = 64
NCORES = 8
WIN = 128
NWIN = 392            # 392 windows of 128 dst nodes
NPAD = NWIN * WIN     # 50176
WPC = NWIN // NCORES  # 49 windows per core
NLOC = WPC * WIN      # 6272 rows per core
NCH = 1               # AllGather chunks per boundary
CHW = WPC // NCH      # 7 windows per chunk
CHROWS = CHW * WIN    # 896 rows per chunk per core

_CACHE = {}


def _perm_rows(s):
    """Global padded node id -> chunk-major table row (chunk, rank, row)."""
    c = s // NLOC
    l = s % NLOC
    k = l // CHROWS
    p = l % CHROWS
    return k * (NCORES * CHROWS) + c * CHROWS + p


def _prep_edges(edge_index):
    """Bucket the E random edges by dst window, pad to uniform tiles.

    Self-loops are NOT included here (they get a dedicated contiguous
    tile per window).  Returns per-core [128, WPC*TR] idx/dlc arrays in
    (partition=slot, col=w*TR+t) layout, plus dinv arrays.
    """
    src = np.asarray(edge_index[0], np.int64)
    dst = np.asarray(edge_index[1], np.int64)
    deg = np.zeros(NPAD, np.float32)
    deg[:N] = np.bincount(dst, minlength=N).astype(np.float32) + 1.0  # + self
    dinv = np.where(deg > 0, 1.0 / np.sqrt(np.maximum(deg, 1.0)), 0.0).astype(
        np.float32
    )

    order = np.argsort(dst, kind="stable")
    src, dst = src[order], dst[order]
    win = (dst >> 7).astype(np.int64)
    cnt = np.bincount(win, minlength=NWIN)
    TR = int(-(-cnt.max() // WIN))

    starts = np.zeros(NWIN + 1, np.int64)
    starts[1:] = np.cumsum(cnt)
    slot = np.arange(len(src), dtype=np.int64) - starts[win]

    p_src = np.zeros((NWIN, TR * WIN), np.int32)
    p_dlc = np.full((NWIN, TR * WIN), 999.0, np.float32)
    flat = win * (TR * WIN) + slot
    p_src.ravel()[flat] = _perm_rows(src).astype(np.int32)
    p_dlc.ravel()[flat] = (dst & 127).astype(np.float32)

    def to_core_layout(a):
        # [NWIN, TR*WIN] -> [c, w, t, p] -> [c, p, w, t] -> [c, 128, WPC*TR]
        a = a.reshape(NCORES, WPC, TR, WIN)
        return np.ascontiguousarray(
            a.transpose(0, 3, 1, 2).reshape(NCORES, WIN, WPC * TR)
        )

    return to_core_layout(p_src), to_core_layout(p_dlc), dinv, TR


def _build_program(TR, K=1):
    from contextlib import ExitStack

    from concourse import bacc, bass, mybir, tile

    f32 = mybir.dt.float32
    bf16 = mybir.dt.bfloat16
    fp8 = mybir.dt.float8e4
    i32 = mybir.dt.int32
    eq = mybir.AluOpType.is_equal
    mul = mybir.AluOpType.mult
    AF = mybir.ActivationFunctionType
    COLS = WPC * TR
    NT = TR + 1  # tiles per window incl. self tile

    nc = bacc.Bacc(
        "TRN2",
        target_bir_lowering=False,
        debug=False,
        enable_asserts=False,
        num_devices=NCORES,
    )

    x_t = nc.dram_tensor("x", [NPAD, F], fp8, kind="ExternalInput")
    xsw_t = nc.dram_tensor("xsw", [WIN, WPC * F], fp8, kind="ExternalInput")
    w0_t = nc.dram_tensor("w0", [F, H], bf16, kind="ExternalInput")
    w1_t = nc.dram_tensor("w1", [H, H], bf16, kind="ExternalInput")
    w2_t = nc.dram_tensor("w2", [H, C], bf16, kind="ExternalInput")
    b0_t = nc.dram_tensor("b0", [WIN, H], f32, kind="ExternalInput")
    b1c_t = nc.dram_tensor("b1c", [H, 1], f32, kind="ExternalInput")
    b2_t = nc.dram_tensor("b2", [WIN, C], f32, kind="ExternalInput")
    idx_t = nc.dram_tensor("idx", [WIN, COLS], i32, kind="ExternalInput")
    dlc_t = nc.dram_tensor("dlc", [WIN, COLS], f32, kind="ExternalInput")
    dloc_t = nc.dram_tensor("dloc", [WIN, WPC], f32, kind="ExternalInput")
    dinvB_t = nc.dram_tensor("dinvB", [WIN, NLOC], f32, kind="ExternalInput")
    out_t = nc.dram_tensor("out", [NLOC, C], f32, kind="ExternalOutput")

    with tile.TileContext(nc) as tc, ExitStack() as ctx:
        dram = ctx.enter_context(tc.tile_pool(name="dram", bufs=1, space="DRAM"))
        loc1 = dram.tile([NLOC, H], fp8, name="loc1")
        locz = dram.tile([NLOC, C], fp8, name="locz")
        # chunked AllGathers write disjoint slices from NCH collectives, which
        # the Shared-DRAM single-writer check rejects -> plain internal DRAM
        # (costs a runtime-internal staging copy, but allows overlap).
        tshared = "Shared" if NCH == 1 else "Local"
        tab1s = [
            dram.tile([NPAD, H], fp8, addr_space=tshared, name=f"tab1_{r}")
            for r in range(K)
        ]
        tabzs = [
            dram.tile([NPAD, C], fp8, addr_space=tshared, name=f"tabz_{r}")
            for r in range(K)
        ]

        const = ctx.enter_context(tc.tile_pool(name="const", bufs=1))

        def load_const(name, src_t, shape, dtype):
            s = const.tile(shape, dtype, name=name)
            nc.sync.dma_start(out=s[:], in_=src_t[:])
            return s

        w0_s = load_const("w0s", w0_t, [F, H], bf16)
        w1_s = load_const("w1s", w1_t, [H, H], bf16)
        w2_s = load_const("w2s", w2_t, [H, C], bf16)
        b0_s = load_const("b0s", b0_t, [WIN, H], f32)
        b1c_s = load_const("b1cs", b1c_t, [H, 1], f32)
        b2_s = load_const("b2s", b2_t, [WIN, C], f32)
        dloc_s = load_const("dlocs", dloc_t, [WIN, WPC], f32)
        dinvB_s = load_const("dinvBs", dinvB_t, [WIN, NLOC], f32)
        xsw_s = load_const("xsws", xsw_t, [WIN, WPC * F], fp8)

        iota_i = const.tile([WIN, WIN], i32, name="iota_i")
        nc.gpsimd.iota(iota_i[:], pattern=[[1, WIN]], base=0, channel_multiplier=0)
        iota_f = const.tile([WIN, WIN], f32, name="iota_f")
        nc.vector.tensor_copy(out=iota_f[:], in_=iota_i[:])
        iota_ci = const.tile([WIN, 1], i32, name="iota_ci")
        nc.gpsimd.iota(iota_ci[:], pattern=[[0, 1]], base=0, channel_multiplier=1)
        iota_cf = const.tile([WIN, 1], f32, name="iota_cf")
        nc.vector.tensor_copy(out=iota_cf[:], in_=iota_ci[:])
        ident = const.tile([WIN, WIN], fp8, name="ident")
        nc.vector.tensor_scalar(
            out=ident[:], in0=iota_f[:], scalar1=iota_cf[:, :1], scalar2=None, op0=eq
        )

        # one-hot sel tiles, built once, reused by every layer (and repeat)
        edge = ctx.enter_context(tc.tile_pool(name="edge", bufs=1))
        sel_all = const.tile([WIN, COLS * WIN], fp8, name="sel_all")

        t1_sb = const.tile([WIN, WPC * H], fp8, name="t1_sb")
        zres = const.tile([WIN, WPC * C], fp8, name="zres")

        gpool = ctx.enter_context(tc.tile_pool(name="gp", bufs=4))
        epil = ctx.enter_context(tc.tile_pool(name="epil", bufs=4))
        psA = ctx.enter_context(tc.tile_pool(name="psA", bufs=3, space="PSUM"))
        psB = ctx.enter_context(tc.tile_pool(name="psB", bufs=3, space="PSUM"))
        psZ = ctx.enter_context(tc.tile_pool(name="psZ", bufs=2, space="PSUM"))

        first = True
        for rep in range(K):
            tab1, tabz = tab1s[rep], tabzs[rep]
            idx_s = edge.tile([WIN, COLS], i32, tag="idx", name=f"idxs_{rep}")
            nc.sync.dma_start(out=idx_s[:], in_=idx_t[:])
            if first:
                first = False
                dlc_s = edge.tile([WIN, COLS], f32, tag="dlc", name="dlcs")
                nc.sync.dma_start(out=dlc_s[:], in_=dlc_t[:])
                for col in range(COLS):
                    nc.vector.tensor_scalar(
                        out=sel_all[:, col * WIN : (col + 1) * WIN],
                        in0=iota_f[:],
                        scalar1=dlc_s[:, col : col + 1],
                        scalar2=None,
                        op0=eq,
                    )

            # ---------- layer 1: gather fp8 x, [F, d] agg, normal epilogue ----
            for w in range(WPC):
                g = gpool.tile([WIN, TR * F], fp8, tag="g", name=f"g1_{rep}_{w}")
                nc.gpsimd.indirect_dma_start(
                    out=g[:],
                    out_offset=None,
                    in_=x_t[:],
                    in_offset=bass.IndirectOffsetOnAxis(
                        ap=idx_s[:, w * TR : (w + 1) * TR], axis=0
                    ),
                )
                agg = psA.tile([F, WIN], f32, tag="agg", name=f"agg1_{rep}_{w}")
                nc.tensor.matmul(
                    out=agg[:],
                    lhsT=xsw_s[:, w * F : (w + 1) * F],
                    rhs=ident[:],
                    start=True,
                    stop=False,
                )
                for t in range(TR):
                    col = w * TR + t
                    nc.tensor.matmul(
                        out=agg[:],
                        lhsT=g[:, t * F : (t + 1) * F],
                        rhs=sel_all[:, col * WIN : (col + 1) * WIN],
                        start=False,
                        stop=(t == TR - 1),
                    )
                aggs = epil.tile([F, WIN], bf16, tag="aggs", name=f"as1_{rep}_{w}")
                nc.scalar.activation(out=aggs[:], in_=agg[:], func=AF.Copy)
                h_ps = psB.tile([WIN, H], f32, tag="h", name=f"h1_{rep}_{w}")
                nc.tensor.matmul(
                    out=h_ps[:], lhsT=aggs[:], rhs=w0_s[:], start=True, stop=True
                )
                ub = epil.tile([WIN, H], f32, tag="ub", name=f"ub1_{rep}_{w}")
                nc.vector.scalar_tensor_tensor(
                    out=ub[:],
                    in0=h_ps[:],
                    scalar=dloc_s[:, w : w + 1],
                    in1=b0_s[:],
                    op0=mul,
                    op1=mybir.AluOpType.add,
                )
                sl = epil.tile([WIN, H], f32, tag="sl", name=f"sl1_{rep}_{w}")
                nc.scalar.activation(out=sl[:], in_=ub[:], func=AF.Silu)
                nc.vector.tensor_scalar(
                    out=t1_sb[:, w * H : (w + 1) * H],
                    in0=sl[:],
                    scalar1=dloc_s[:, w : w + 1],
                    scalar2=None,
                    op0=mul,
                )
                nc.sync.dma_start(
                    out=loc1[w * WIN : (w + 1) * WIN, :],
                    in_=t1_sb[:, w * H : (w + 1) * H],
                )
                if w % CHW == CHW - 1:
                    k = w // CHW
                    nc.gpsimd.collective_compute(
                        "AllGather",
                        mybir.AluOpType.bypass,
                        replica_groups=[list(range(NCORES))],
                        ins=[loc1[k * CHROWS : (k + 1) * CHROWS, :].opt()],
                        outs=[
                            tab1[
                                k * NCORES * CHROWS : (k + 1) * NCORES * CHROWS, :
                            ].opt()
                        ],
                    )

            # ---------- layer 2: gather fp8 t1, transposed epilogue, z ----
            for w in range(WPC):
                g = gpool.tile([WIN, TR * F], fp8, tag="g", name=f"g2_{rep}_{w}")
                nc.gpsimd.indirect_dma_start(
                    out=g[:],
                    out_offset=None,
                    in_=tab1[:],
                    in_offset=bass.IndirectOffsetOnAxis(
                        ap=idx_s[:, w * TR : (w + 1) * TR], axis=0
                    ),
                )
                agg = psA.tile([F, WIN], f32, tag="agg", name=f"agg2_{rep}_{w}")
                nc.tensor.matmul(
                    out=agg[:],
                    lhsT=t1_sb[:, w * H : (w + 1) * H],
                    rhs=ident[:],
                    start=True,
                    stop=False,
                )
                for t in range(TR):
                    col = w * TR + t
                    nc.tensor.matmul(
                        out=agg[:],
                        lhsT=g[:, t * F : (t + 1) * F],
                        rhs=sel_all[:, col * WIN : (col + 1) * WIN],
                        start=False,
                        stop=(t == TR - 1),
                    )
                aggs = epil.tile([F, WIN], bf16, tag="aggs", name=f"as2_{rep}_{w}")
                nc.scalar.activation(out=aggs[:], in_=agg[:], func=AF.Copy)
                # transposed dense: hT[fo, d] = sum_f W1[f, fo] * aggs[f, d]
                hT_ps = psB.tile([H, WIN], f32, tag="h", name=f"h2_{rep}_{w}")
                nc.tensor.matmul(
                    out=hT_ps[:], lhsT=w1_s[:], rhs=aggs[:], start=True, stop=True
                )
                ua = epil.tile([H, WIN], f32, tag="ua", name=f"ua2_{rep}_{w}")
                nc.vector.tensor_mul(
                    out=ua[:],
                    in0=hT_ps[:],
                    in1=dinvB_s[:, w * WIN : (w + 1) * WIN],
                )
                sl = epil.tile([H, WIN], f32, tag="sl", name=f"sl2_{rep}_{w}")
                nc.scalar.activation(
                    out=sl[:], in_=ua[:], func=AF.Silu, bias=b1c_s[:, :1], scale=1.0
                )
                t2T = epil.tile([H, WIN], bf16, tag="t2", name=f"t2_{rep}_{w}")
                nc.vector.tensor_mul(
                    out=t2T[:],
                    in0=sl[:],
                    in1=dinvB_s[:, w * WIN : (w + 1) * WIN],
                )
                z_ps = psZ.tile([WIN, C], f32, tag="z", name=f"z_{rep}_{w}")
                nc.tensor.matmul(
                    out=z_ps[:], lhsT=t2T[:], rhs=w2_s[:], start=True, stop=True
                )
                nc.scalar.activation(
                    out=zres[:, w * C : (w + 1) * C], in_=z_ps[:], func=AF.Copy
                )
                nc.sync.dma_start(
                    out=locz[w * WIN : (w + 1) * WIN, :],
                    in_=zres[:, w * C : (w + 1) * C],
                )
                if w % CHW == CHW - 1:
                    k = w // CHW
                    nc.gpsimd.collective_compute(
                        "AllGather",
                        mybir.AluOpType.bypass,
                        replica_groups=[list(range(NCORES))],
                        ins=[locz[k * CHROWS : (k + 1) * CHROWS, :].opt()],
                        outs=[
                            tabz[
                                k * NCORES * CHROWS : (k + 1) * NCORES * CHROWS, :
                            ].opt()
                        ],
                    )

            # ---------- layer 3: gather fp8 z (64 cols), [d, c] agg ----
            for w in range(WPC):
                g = gpool.tile([WIN, TR * C], fp8, tag="g3", name=f"g3_{rep}_{w}")
                nc.gpsimd.indirect_dma_start(
                    out=g[:],
                    out_offset=None,
                    in_=tabz[:],
                    in_offset=bass.IndirectOffsetOnAxis(
                        ap=idx_s[:, w * TR : (w + 1) * TR], axis=0
                    ),
                )
                agg = psZ.tile([WIN, C], f32, tag="z", name=f"agg3_{rep}_{w}")
                nc.tensor.matmul(
                    out=agg[:],
                    lhsT=ident[:],
                    rhs=zres[:, w * C : (w + 1) * C],
                    start=True,
                    stop=False,
                )
                for t in range(TR):
                    col = w * TR + t
                    nc.tensor.matmul(
                        out=agg[:],
                        lhsT=sel_all[:, col * WIN : (col + 1) * WIN],
                        rhs=g[:, t * C : (t + 1) * C],
                        start=False,
                        stop=(t == TR - 1),
                    )
                ls = epil.tile([WIN, C], f32, tag="ls", name=f"ls_{rep}_{w}")
                nc.vector.scalar_tensor_tensor(
                    out=ls[:],
                    in0=agg[:],
                    scalar=dloc_s[:, w : w + 1],
                    in1=b2_s[:],
                    op0=mul,
                    op1=mybir.AluOpType.add,
                )
                mx = epil.tile([WIN, 1], f32, tag="mx", name=f"mx_{rep}_{w}")
                nc.vector.tensor_reduce(
                    out=mx[:],
                    in_=ls[:],
                    axis=mybir.AxisListType.X,
                    op=mybir.AluOpType.max,
                )
                nmx = epil.tile([WIN, 1], f32, tag="nmx", name=f"nmx_{rep}_{w}")
                nc.vector.tensor_scalar_mul(nmx[:], mx[:], -1.0)
                ex = epil.tile([WIN, C], f32, tag="ex", name=f"ex_{rep}_{w}")
                sm = epil.tile([WIN, 1], f32, tag="sm", name=f"sm_{rep}_{w}")
                nc.scalar.activation(
                    out=ex[:],
                    in_=ls[:],
                    func=AF.Exp,
                    bias=nmx[:, :1],
                    scale=1.0,
                    accum_out=sm[:],
                )
                lg = epil.tile([WIN, 1], f32, tag="lg", name=f"lg_{rep}_{w}")
                nc.scalar.activation(out=lg[:], in_=sm[:], func=AF.Ln)
                lse = epil.tile([WIN, 1], f32, tag="lse", name=f"lse_{rep}_{w}")
                nc.vector.tensor_add(out=lse[:], in0=lg[:], in1=mx[:])
                o = epil.tile([WIN, C], f32, tag="o", name=f"o_{rep}_{w}")
                nc.vector.tensor_scalar_sub(o[:], ls[:], lse[:, :1])
                nc.sync.dma_start(out=out_t[w * WIN : (w + 1) * WIN, :], in_=o[:])

    nc.compile()
    return nc


def _get_program(TR, K=1):
    key = (TR, K, NCH)
    if key not in _CACHE:
        _CACHE[key] = _build_program(TR, K)
    return _CACHE[key]


def _prep_inputs(x, edge_index, W0, b0, W1, b1, W2, b2):
    import ml_dtypes

    fp8 = ml_dtypes.float8_e4m3
    x = np.asarray(x, np.float32)
    idx, dlc, dinv, TR = _prep_edges(np.asarray(edge_index))

    x_pad = np.zeros((NPAD, F), np.float32)
    x_pad[:N] = x
    t0 = dinv[:, None] * x_pad
    t0_perm = np.empty((NPAD, F), fp8)
    t0_perm[_perm_rows(np.arange(NPAD))] = t0.astype(fp8)
    # pre-swizzled self table: xsw[c, p, w*F:(w+1)*F] = t0[c, w*128+p, :]
    t0_loc = t0.astype(fp8).reshape(NCORES, WPC, WIN, F)
    xsw = np.ascontiguousarray(t0_loc.transpose(0, 2, 1, 3).reshape(NCORES, WIN, WPC * F))

    dinv_loc = dinv.reshape(NCORES, WPC, WIN)  # [c, w, d]
    dloc = np.ascontiguousarray(dinv_loc.transpose(0, 2, 1))  # [c, d, w]
    dinvB = np.ascontiguousarray(
        np.broadcast_to(
            dinv.reshape(NCORES, 1, NLOC), (NCORES, WIN, NLOC)
        ).astype(np.float32)
    )

    common = {
        "x": t0_perm,
        "w0": np.asarray(W0, np.float32).astype(ml_dtypes.bfloat16),
        "w1": np.asarray(W1, np.float32).astype(ml_dtypes.bfloat16),
        "w2": np.asarray(W2, np.float32).astype(ml_dtypes.bfloat16),
        "b0": np.broadcast_to(np.asarray(b0, np.float32), (WIN, H)).copy(),
        "b1c": np.asarray(b1, np.float32).reshape(H, 1).copy(),
        "b2": np.broadcast_to(np.asarray(b2, np.float32), (WIN, C)).copy(),
    }
    in_maps = [
        dict(
            common,
            xsw=xsw[c],
            idx=idx[c],
            dlc=dlc[c],
            dloc=dloc[c],
            dinvB=dinvB[c],
        )
        for c in range(NCORES)
    ]
    return in_maps, TR


def kernel(x, edge_index, W0, b0, W1, b1, W2, b2, **_):
    from concourse.bass_utils import run_bass_kernel_spmd

    in_maps, TR = _prep_inputs(x, edge_index, W0, b0, W1, b1, W2, b2)
    nc = _get_program(TR)
    res = run_bass_kernel_spmd(nc, in_maps, list(range(NCORES)))
    out = np.concatenate(
        [np.asarray(res.results[c]["out"]) for c in range(NCORES)], axis=0
    )
    return out[:N]
